# revision 6
# baseline (speedup 1.0000x reference)
"""Multi-head causal attention (B=1, S=2048, E=2048, H=16, DH=128) on 8 TRN2
NeuronCores.

Sharding: tensor-parallel over heads; core c owns heads 2c and 2c+1; output
projection column-sharded (core c computes y[:, 256c:256(c+1)]) after an
AllGather of the per-group attention outputs.

v2 schedule (vs v1): all-bf16 operands, fast reciprocal, the last q-group is
split into two 256-query subgroups so the final AllGather starts earlier and
is half the size, output-projection tails run after all attention filling the
PE during the trailing gathers, og tiles are prefetched with one batched DMA
per group issued from gpsimd/scalar so the Sync queue stays clear, and PSUM
is budgeted so consecutive attention groups never alias mid-flight banks
(psS 2x[128,1024] + psO 3 slots + psN 1 shared bank = 8 banks).

attention(group): S^T = K @ Q^T per 128-key block, exp on ScalarE,
block-causal mask post-exp on DVE, denominators via ones-column matmuls,
normalization via fast reciprocal + gpsimd partition-broadcast + DVE mult.
All PSUM accumulation fp32.
"""
import os
import sys

if "/opt/trn_rl_repo" not in sys.path:
    sys.path.insert(0, "/opt/trn_rl_repo")

import numpy as np

B, S, E, H = 1, 2048, 2048, 16
DH = E // H          # 128
N_CORES = 8
HPC = H // N_CORES   # heads per core = 2
KT = E // 128        # 16 contraction tiles
QG = 512             # base q-group width
SBK = S // 128       # 16 s/sk blocks
CSL = E // N_CORES   # 256 output columns per core

# (q0, qw, nj): query start, width, number of 128-key blocks attended
GROUPS = [(0, 512, 4), (512, 512, 8), (1024, 512, 12),
          (1536, 256, 14), (1792, 256, 16)]
NG = len(GROUPS)

_CACHE = {}


def _build(fp_name: str):
    import concourse.bass as bass  # noqa: F401
    import concourse.mybir as mybir
    import concourse.tile as tile
    from concourse import bacc

    FP = getattr(mybir.dt, fp_name)
    F32 = mybir.dt.float32
    BF16 = mybir.dt.bfloat16
    AF = mybir.ActivationFunctionType

    nc = bacc.Bacc("TRN2", target_bir_lowering=False, debug=False,
                   num_devices=N_CORES)

    xT_t = nc.dram_tensor("xT", [E, S], BF16, kind="ExternalInput")
    wq_t = nc.dram_tensor("wq", [128, KT * HPC * DH], BF16, kind="ExternalInput")
    wk_t = nc.dram_tensor("wk", [128, KT * HPC * DH], BF16, kind="ExternalInput")
    wv_t = nc.dram_tensor("wv", [128, KT * HPC * DH], BF16, kind="ExternalInput")
    bq_t = nc.dram_tensor("bq", [DH, HPC], F32, kind="ExternalInput")
    bk_t = nc.dram_tensor("bk", [DH, HPC], F32, kind="ExternalInput")
    bv_t = nc.dram_tensor("bv", [1, HPC * DH], F32, kind="ExternalInput")
    wo_t = nc.dram_tensor("wo", [128, KT * CSL], BF16, kind="ExternalInput")
    bo_t = nc.dram_tensor("bo", [1, CSL], F32, kind="ExternalInput")
    mask_t = nc.dram_tensor("mask", [4 * 128, QG], BF16, kind="ExternalInput")
    y_t = nc.dram_tensor("y", [S, CSL], F32, kind="ExternalOutput")

    xT_r = xT_t.ap().rearrange("(kt p) s -> kt p s", p=128)
    mask_r = mask_t.ap().rearrange("(jm p) q -> jm p q", p=128)

    scale = 1.0 / float(np.sqrt(DH))

    with tile.TileContext(nc) as tc:
        with tc.tile_pool(name="const", bufs=1) as constp, \
             tc.tile_pool(name="prod", bufs=1) as prodp, \
             tc.tile_pool(name="dram", bufs=1, space="DRAM") as dramp:
            # head-0 Q/K weights first: they gate the first matmul.
            # Quarter-granularity so the first chain starts after 64KB.
            wqk_sb = {}
            for nm_ in ("wq", "wk"):
                for hh in range(HPC):
                    wqk_sb[(nm_, hh)] = constp.tile(
                        [128, KT * DH], BF16,
                        tag=f"w_{nm_}{hh}", name=f"w_{nm_}{hh}")
            QKD = KT * DH // 4
            for quarter in range(4):
                for nm_, t_ in (("wq", wq_t), ("wk", wk_t)):
                    nc.sync.dma_start(
                        wqk_sb[(nm_, 0)][:, quarter * QKD:(quarter + 1) * QKD],
                        t_.ap()[:, quarter * QKD:(quarter + 1) * QKD])
            bqs = constp.tile([DH, HPC], F32)
            nc.sync.dma_start(bqs[:], bq_t.ap()[:])
            bks = constp.tile([DH, HPC], F32)
            nc.sync.dma_start(bks[:], bk_t.ap()[:])
            ones_f32 = constp.tile([128, 128], F32)
            nc.vector.memset(ones_f32[:], 1.0)
            ones_col = constp.tile([128, 1], FP)
            nc.vector.tensor_copy(ones_col[:], ones_f32[:, 0:1])
            bvs = constp.tile([128, HPC * DH], F32)
            bos = constp.tile([128, CSL], F32)
            masks = constp.tile([128, 4 * QG], BF16)
            wv_sb = constp.tile([128, KT * HPC * DH], BF16, tag="wv_sb",
                                name="wv_sb")
            wos = constp.tile([128, KT * CSL], BF16, tag="wos", name="wos")

            # --- products ---
            qkt = prodp.tile([128, HPC * S], FP)   # Q^T, head hh at cols hh*S
            kkt = prodp.tile([128, HPC * S], FP)   # K^T
            vt = prodp.tile([128, SBK * HPC * DH], FP)  # V, s-block sb at sb*256

            cin = [dramp.tile([HPC * DH, qw], BF16, tag=f"cin{g}",
                              name=f"cin{g}")
                   for g, (q0, qw, nj) in enumerate(GROUPS)]
            cout = [dramp.tile([N_CORES, HPC * DH, qw], BF16,
                               tag=f"cout{g}", name=f"cout{g}",
                               addr_space="Shared")
                    for g, (q0, qw, nj) in enumerate(GROUPS)]

            with tc.tile_pool(name="osb", bufs=1) as osbp, \
                 tc.tile_pool(name="pt", bufs=8) as ptp, \
                 tc.tile_pool(name="rec", bufs=2) as recp, \
                 tc.tile_pool(name="bcs", bufs=2) as bcsp:
                o_sbuf = osbp.tile([128, HPC * S], BF16)

                def attn(gi):
                    q0, qw, nj = GROUPS[gi]
                    npairs = nj // 2
                    jmax = nj - 1
                    mask_start = q0 // 128     # first masked j-block
                    jm0 = (512 * (q0 // 512)) // 128
                    col_off = q0 - 512 * (q0 // 512)
                    with tc.tile_pool(name=f"psS{gi}", bufs=3,
                                      space="PSUM") as psS, \
                         tc.tile_pool(name=f"psO{gi}", bufs=3,
                                      space="PSUM") as psO, \
                         tc.tile_pool(name=f"psN{gi}", bufs=2,
                                      space="PSUM") as psN:
                        o_acc = [psO.tile([128, QG], F32, tag="o",
                                          name=f"o{hh}") for hh in range(HPC)]
                        s_acc = [psN.tile([1, QG], F32, tag="n",
                                          name=f"n{hh}") for hh in range(HPC)]

                        def emit_pv(hh, jp, pt):
                            for dj in range(2):
                                j = 2 * jp + dj
                                nc.tensor.matmul(
                                    o_acc[hh][:, 0:qw],
                                    vt[:, j * HPC * DH + hh * DH:
                                       j * HPC * DH + (hh + 1) * DH],
                                    pt[:, dj * qw:(dj + 1) * qw],
                                    start=(j == 0), stop=(j == jmax))
                                nc.tensor.matmul(
                                    s_acc[hh][:, 0:qw],
                                    ones_col[:],
                                    pt[:, dj * qw:(dj + 1) * qw],
                                    start=(j == 0), stop=(j == jmax))

                        pend = []
                        for jp in range(npairs):
                            for hh in range(HPC):
                                pt = ptp.tile([128, 2 * QG], FP, tag="p",
                                              name="pt")
                                for dj in range(2):
                                    j = 2 * jp + dj
                                    ps = psS.tile([128, QG], F32, tag="s",
                                                  name="ps")
                                    nc.tensor.matmul(
                                        ps[:, 0:qw],
                                        kkt[:, hh * S + j * 128:
                                            hh * S + (j + 1) * 128],
                                        qkt[:, hh * S + q0:
                                            hh * S + q0 + qw],
                                        start=True, stop=True)
                                    nc.scalar.activation(
                                        pt[:, dj * qw:(dj + 1) * qw],
                                        ps[:, 0:qw], AF.Exp, scale=scale)
                                    if j >= mask_start:
                                        jm = j - jm0
                                        nc.vector.tensor_mul(
                                            pt[:, dj * qw:(dj + 1) * qw],
                                            pt[:, dj * qw:(dj + 1) * qw],
                                            masks[:, jm * QG + col_off:
                                                  jm * QG + col_off + qw])
                                pend.append((hh, jp, pt))
                                while len(pend) > 3:
                                    emit_pv(*pend.pop(0))
                        while pend:
                            emit_pv(*pend.pop(0))

                        for hh in range(HPC):
                            rec = recp.tile([1, QG], F32, tag="r", name="rec")
                            nc.vector.reciprocal_approx_fast(
                                rec[:, 0:qw], s_acc[hh][:, 0:qw])
                            bcs = bcsp.tile([128, QG], F32, tag="b",
                                            name="bcs")
                            nc.gpsimd.partition_broadcast(bcs[:, 0:qw],
                                                          rec[:, 0:qw])
                            nc.vector.tensor_mul(
                                o_sbuf[:, hh * S + q0:hh * S + q0 + qw],
                                o_acc[hh][:, 0:qw], bcs[:, 0:qw])
                            nc.sync.dma_start(
                                cin[gi].rearrange("(hh p) q -> hh p q",
                                                  p=128)[hh],
                                o_sbuf[:, hh * S + q0:hh * S + q0 + qw])
                    nc.gpsimd.collective_compute(
                        "AllGather",
                        mybir.AluOpType.bypass,
                        replica_groups=[list(range(N_CORES))],
                        ins=[cin[gi].opt()],
                        outs=[cout[gi].opt()],
                    )

                # ===== projections =====
                with tc.tile_pool(name="xt", bufs=1) as xtp:
                    xt = xtp.tile([128, KT * S], BF16)

                    def proj_qk(hh, stream=False, tag=""):
                        with tc.tile_pool(name=f"psP{tag}", bufs=1,
                                          space="PSUM") as psA:
                            specs = [("wq", qkt, bqs), ("wk", kkt, bks)]
                            accs = {p: {g: psA.tile([128, QG], F32, tag="qk",
                                                    name=f"qk{tag}_{p}{g}",
                                                    bufs=8)
                                        for g in range(4)} for p in range(2)}
                            if stream:
                                # kt-major: chase the x^T stream
                                for kt in range(KT):
                                    nc.sync.dma_start(
                                        xt[:, kt * S:(kt + 1) * S], xT_r[kt])
                                    for p, (wn, prod, bias) in enumerate(specs):
                                        wtile = wqk_sb[(wn, hh)][:, kt * DH:
                                                                 (kt + 1) * DH]
                                        for g in range(4):
                                            nc.tensor.matmul(
                                                accs[p][g][:], wtile,
                                                xt[:, kt * S + g * QG:
                                                   kt * S + (g + 1) * QG],
                                                start=(kt == 0),
                                                stop=(kt == KT - 1))
                            else:
                                # group-major: drain overlaps next chain
                                for p, (wn, prod, bias) in enumerate(specs):
                                    for g in range(4):
                                        for kt in range(KT):
                                            wtile = wqk_sb[(wn, hh)][
                                                :, kt * DH:(kt + 1) * DH]
                                            nc.tensor.matmul(
                                                accs[p][g][:], wtile,
                                                xt[:, kt * S + g * QG:
                                                   kt * S + (g + 1) * QG],
                                                start=(kt == 0),
                                                stop=(kt == KT - 1))
                                        nc.scalar.activation(
                                            prod[:, hh * S + g * QG:
                                                 hh * S + (g + 1) * QG],
                                            accs[p][g][:], AF.Identity,
                                            bias=bias[:, hh:hh + 1])
                            if stream:
                                for p, (wn, prod, bias) in enumerate(specs):
                                    for g in range(4):
                                        nc.scalar.activation(
                                            prod[:, hh * S + g * QG:
                                                 hh * S + (g + 1) * QG],
                                            accs[p][g][:], AF.Identity,
                                            bias=bias[:, hh:hh + 1])

                    def proj_v(blocks, tag=""):
                        with tc.tile_pool(name=f"psV{tag}", bufs=1,
                                          space="PSUM") as psA:
                            v_accs = {b: psA.tile([128, HPC * DH], F32,
                                                  tag="v", name=f"v{tag}_{b}",
                                                  bufs=len(blocks))
                                      for b in blocks}
                            for b in blocks:
                                for kt in range(KT):
                                    nc.tensor.matmul(
                                        v_accs[b][:],
                                        xt[:, kt * S + b * 128:
                                           kt * S + (b + 1) * 128],
                                        wv_sb[:, kt * HPC * DH:
                                              (kt + 1) * HPC * DH],
                                        start=(kt == 0), stop=(kt == KT - 1))
                                nc.vector.tensor_add(
                                    vt[:, b * HPC * DH:(b + 1) * HPC * DH],
                                    v_accs[b][:], bvs[:])

                    proj_qk(0, stream=True, tag="s1")
                    # remaining weights, ordered by first use
                    for nm_, t_ in (("wq", wq_t), ("wk", wk_t)):
                        nc.sync.dma_start(wqk_sb[(nm_, 1)][:],
                                          t_.ap()[:, KT * DH:2 * KT * DH])
                    nc.sync.dma_start(wv_sb[:], wv_t.ap()[:])
                    nc.sync.dma_start(
                        bvs[:], bv_t.ap().to_broadcast((128, HPC * DH)))
                    for jm in range(4):
                        nc.sync.dma_start(masks[:, jm * QG:(jm + 1) * QG],
                                          mask_r[jm])
                    nc.sync.dma_start(wos[:], wo_t.ap()[:])
                    nc.sync.dma_start(bos[:],
                                      bo_t.ap().to_broadcast((128, CSL)))
                    proj_qk(1, tag="s2")
                    proj_v([0, 1, 2, 3, 4, 5, 6, 7], tag="s3")
                    attn(0)
                    attn(1)
                    proj_v([8, 9, 10, 11, 12, 13, 14, 15], tag="s4")
                # xt pool closed: 64KB/partition freed for og prefetch
                with tc.tile_pool(name="og", bufs=5) as ogp:
                    og = {}

                    def og_load(gi, engine):
                        q0, qw, nj = GROUPS[gi]
                        t = ogp.tile([128, KT * QG], BF16, tag="og",
                                     name=f"og{gi}")
                        og[gi] = t
                        engine.dma_start(
                            t[:, 0:KT * qw].rearrange("p (c h q) -> p c h q",
                                                      c=N_CORES, h=HPC),
                            cout[gi].rearrange("c (h p) q -> p c h q",
                                               p=128))

                    attn(2)
                    og_load(0, nc.gpsimd)   # AG0 done long before this point
                    attn(3)
                    og_load(1, nc.gpsimd)
                    attn(4)
                    # scalar is idle after the last exp; waits are harmless
                    og_load(2, nc.scalar)
                    og_load(3, nc.scalar)
                    og_load(4, nc.scalar)

                    # ===== tail: output projection, column-sharded =====
                    with tc.tile_pool(name="yst", bufs=2) as ystp, \
                         tc.tile_pool(name="psY", bufs=4,
                                      space="PSUM") as psY:
                        for gi, (q0, qw, nj) in enumerate(GROUPS):
                            nsb = qw // 128
                            yst = ystp.tile([128, 4 * CSL], F32, tag="ys",
                                            name=f"yst{gi}")
                            for i in range(nsb):
                                acc = psY.tile([128, CSL], F32, tag="y",
                                               name="yacc")
                                for kt in range(KT):
                                    nc.tensor.matmul(
                                        acc[:],
                                        og[gi][:, kt * qw + i * 128:
                                               kt * qw + (i + 1) * 128],
                                        wos[:, kt * CSL:(kt + 1) * CSL],
                                        start=(kt == 0), stop=(kt == KT - 1))
                                nc.vector.tensor_add(
                                    yst[:, i * CSL:(i + 1) * CSL],
                                    acc[:], bos[:])
                            nc.sync.dma_start(
                                y_t.ap()[q0:q0 + qw, :].rearrange(
                                    "(sb p) c -> p sb c", p=128),
                                yst[:, 0:nsb * CSL].rearrange(
                                    "p (sb c) -> p sb c", c=CSL))

    nc.compile()
    return nc


def _tilize(w):
    """[E, cols] -> [128, KT*cols]: k-tile kt at columns kt*cols."""
    cols = w.shape[1]
    return np.ascontiguousarray(
        w.reshape(KT, 128, cols).transpose(1, 0, 2).reshape(128, KT * cols))


def _tilize_hm(w):
    """[E, HPC*DH] -> [128, HPC*KT*DH], head-major then k-tile."""
    return np.ascontiguousarray(
        w.reshape(KT, 128, HPC, DH).transpose(1, 2, 0, 3)
        .reshape(128, HPC * KT * DH))


def _prep_inputs(x, Wq, bq, Wk, bk, Wv, bv, WO, bo):
    import ml_dtypes

    f32 = np.float32
    bf16 = ml_dtypes.bfloat16
    xT = np.ascontiguousarray(np.asarray(x, f32)[0].T).astype(bf16)
    Wq = np.asarray(Wq, f32); Wk = np.asarray(Wk, f32); Wv = np.asarray(Wv, f32)
    bq = np.asarray(bq, f32); bk = np.asarray(bk, f32); bv = np.asarray(bv, f32)
    WO = np.asarray(WO, f32); bo = np.asarray(bo, f32)

    jm = np.arange(4)[:, None, None]
    r = np.arange(128)[None, :, None]
    c = np.arange(QG)[None, None, :]
    mask = (128 * jm + r <= c).astype(bf16).reshape(4 * 128, QG)

    in_maps = []
    for cidx in range(N_CORES):
        h0, h1 = HPC * cidx, HPC * cidx + 1
        in_maps.append({
            "xT": xT,
            "wq": _tilize_hm(np.concatenate([Wq[h0], Wq[h1]], 1)).astype(bf16),
            "wk": _tilize_hm(np.concatenate([Wk[h0], Wk[h1]], 1)).astype(bf16),
            "wv": _tilize(np.concatenate([Wv[h0], Wv[h1]], 1)).astype(bf16),
            "bq": np.ascontiguousarray(np.stack([bq[h0], bq[h1]], 1)),
            "bk": np.ascontiguousarray(np.stack([bk[h0], bk[h1]], 1)),
            "bv": np.concatenate([bv[h0], bv[h1]])[None, :].copy(),
            "wo": _tilize(np.ascontiguousarray(
                WO[:, CSL * cidx:CSL * (cidx + 1)])).astype(bf16),
            "bo": bo[CSL * cidx:CSL * (cidx + 1)][None, :].copy(),
            "mask": mask,
        })
    return in_maps


def kernel(x, Wq, bq, Wk, bk, Wv, bv, WO, bo, trace=False,
           fp_name="bfloat16"):
    from concourse.bass_utils import run_bass_kernel_spmd

    key = fp_name
    if key not in _CACHE:
        _CACHE[key] = _build(fp_name)
    nc = _CACHE[key]

    in_maps = _prep_inputs(x, Wq, bq, Wk, bk, Wv, bv, WO, bo)
    kwargs = {}
    if trace:
        kwargs["trace"] = True
    res = run_bass_kernel_spmd(nc, in_maps, core_ids=list(range(N_CORES)),
                               **kwargs)
    kernel.last_results = res

    y = np.concatenate([res.results[c]["y"] for c in range(N_CORES)], axis=1)
    return y.reshape(B, S, E).astype(np.float32)
